# revision 12
# baseline (speedup 1.0000x reference)
"""Trainium2 Bass kernel for nn_CrossVariableMixingConv.

Reference computation (per row of x, B*L rows, C=862 channels):
    h   = conv1d(x, Wup, k=7, pad=3) + bup      # (RANK=8 channels)
    g   = gelu(h)  (erf-exact)
    d   = sum_r Wdown[r] * g[r] + bdown
    y   = LayerNorm(x + d) * gamma + beta       # LN over C

Sharding: pure data parallelism - the B*L = 11520 rows are split into 8
shards of 1440 rows, one per NeuronCore; the ~70 conv params are
replicated.

Per-core kernel structure (rows on partitions, chunks of 128 rows):
  - The k=7 conv is a bf16 matmul with the data stationary: lhsT = a
    host-pretransposed tap-slice [128 taps, rows] per window (8 windows
    of 108 output cols, partition-major in DRAM so each load is a clean
    2880B-per-line DMA), rhs = a banded weight matrix [128, 432]
    covering 4 ranks x 108 cols; a baked ones-row adds bup in-matmul.
  - Gelu (erf-exact LUT) runs on the Scalar engine out of PSUM writing
    fp8(e4m3) G in SBUF.  The Scalar engine is the critical path
    (1 elem/cycle/lane x 8 ranks x C), so everything else stays off it.
  - The rank contraction (Wdown) is 4 accumulating fp8 DoubleRow
    matmuls per half of C - each contracts TWO ranks per streamed
    column (out = sum_s W[:,s].T @ I[:,s]) against scaled fp8
    identities - halving the Tensor-engine column count.
  - The down-projection of chunk N is issued AFTER chunk N+1's conv
    matmuls (software pipelining), so the Scalar engine never waits for
    the PE queue to drain a down-projection.
  - Residual + LayerNorm run on the Vector engine in bf16
    (bn_stats/bn_aggr, mult-only Newton rstd, fused (y-mu)*rstd);
    results DMA out in bf16.

bdown is dropped: LayerNorm is invariant to a constant shift per row.
gamma/beta are applied only when not identity (ones/zeros here).
"""

import sys

for _p in ("/opt/trn_rl_repo",):
    if _p not in sys.path:
        sys.path.insert(0, _p)

import numpy as np
import ml_dtypes

B, L, C = 16, 720, 862
RANK, KTAPS = 8, 7
NCORES = 8
ROWS = B * L                 # 11520
RPC = ROWS // NCORES         # 1440 rows per core
PCH = 128                    # rows per chunk (partition dim)
NCHUNK = (RPC + PCH - 1) // PCH   # 12 (11 full + 1 of 32)
WW = 108                     # conv output columns per window
NW = 8                       # windows: 8*108 = 864 >= 862
CP = NW * WW                 # 864 padded output columns
CHALF = C // 2               # 431
EPS = 1e-5

_CACHE: dict = {}


def _build(apply_gamma_beta: bool):
    """Build + compile the per-core Bass program. Cached per flag."""
    key = ("nc", apply_gamma_beta)
    if key in _CACHE:
        return _CACHE[key]

    from contextlib import ExitStack

    import concourse.bacc as bacc
    import concourse.bass as bass
    import concourse.tile as tile
    from concourse import mybir

    f32 = mybir.dt.float32
    bf16 = mybir.dt.bfloat16
    fp8 = mybir.dt.float8e4
    AF = mybir.ActivationFunctionType
    ALU = mybir.AluOpType
    DR = mybir.MatmulPerfMode.DoubleRow

    nc = bacc.Bacc(
        "TRN2", target_bir_lowering=False, debug=False, num_devices=NCORES
    )

    xw_d = nc.dram_tensor("xw", [NW, 128, RPC], bf16, kind="ExternalInput").ap()
    xb_d = nc.dram_tensor("xb", [RPC, C], bf16, kind="ExternalInput").ap()
    band_d = nc.dram_tensor("band", [128, 2, 4 * WW], bf16, kind="ExternalInput").ap()
    wdi_d = nc.dram_tensor("wdi", [128, RANK, 128], bf16, kind="ExternalInput").ap()
    if apply_gamma_beta:
        gb_d = nc.dram_tensor("gb", [2, C], f32, kind="ExternalInput").ap()
    y_d = nc.dram_tensor("y", [RPC, C], bf16, kind="ExternalOutput").ap()

    with tile.TileContext(nc) as tc, ExitStack() as ctx:
        singles = ctx.enter_context(tc.tile_pool(name="singles", bufs=1))
        xp = ctx.enter_context(tc.tile_pool(name="xin", bufs=2))
        gp = ctx.enter_context(tc.tile_pool(name="g", bufs=3))
        op = ctx.enter_context(tc.tile_pool(name="o", bufs=3))
        stp = ctx.enter_context(tc.tile_pool(name="st", bufs=3))
        hp = ctx.enter_context(tc.tile_pool(name="hps", bufs=3, space="PSUM"))
        dp = ctx.enter_context(tc.tile_pool(name="dps", bufs=1, space="PSUM"))

        # Tiny weight tensors first so the first conv matmul waits only
        # on its own window slice, not the whole input stream.
        band_t = singles.tile([128, 2, 4 * WW], bf16)
        nc.sync.dma_start(out=band_t, in_=band_d)
        wdi_t = singles.tile([128, RANK, 128], bf16)
        nc.gpsimd.dma_start(out=wdi_t, in_=wdi_d)
        # Window tap-slices [128 taps, rows], partition-major loads
        # (2880B per line); partition 127 is the baked ones row.
        # Each split in two so chunk 0's convs start as soon as the first
        # 256 row-columns land instead of after the full 2.95MB stream.
        xtw = []
        for w in range(NW):
            t = singles.tile([128, RPC], bf16, tag=f"xtw{w}")
            eng = nc.sync if w % 2 == 0 else nc.gpsimd
            eng.dma_start(out=t[:, 0:256], in_=xw_d[w, :, 0:256])
            eng.dma_start(out=t[:, 256:RPC], in_=xw_d[w, :, 256:RPC])
            xtw.append(t)
        if apply_gamma_beta:
            gamma_rep = singles.tile([128, C], f32)
            beta_rep = singles.tile([128, C], f32)
            for rep, row in ((gamma_rep, 0), (beta_rep, 1)):
                src = bass.AP(
                    tensor=gb_d.tensor,
                    offset=gb_d.offset + row * C,
                    ap=[[0, 128], [1, C]],
                )
                nc.gpsimd.dma_start(out=rep, in_=src)

        def emit_tail(G, xb_t, dT, n0, nr):
            """Down-projection + residual/LN + store for a finished chunk.

            r-outer so each wdi_r stationary is loaded once (2 matmuls
            per load) - per-matmul LDWEIGHTS is the dominant PE tax."""
            for r in range(RANK):
                for hh in range(2):
                    nc.tensor.matmul(
                        dT[:nr, hh, 0:CHALF],
                        lhsT=wdi_t[:nr, r, :nr],
                        rhs=G[:nr, r, hh * CHALF : (hh + 1) * CHALF],
                        start=(r == 0),
                        stop=(r == RANK - 1),
                    )

            y_t = op.tile([128, C], bf16, tag="y")
            yc = y_t[:nr].rearrange("p (s c) -> p s c", s=2)
            nc.vector.tensor_add(
                out=yc,
                in0=xb_t[:nr].rearrange("p (s c) -> p s c", s=2),
                in1=dT[:nr, :, 0:CHALF],
            )

            st = stp.tile([128, 2, 6], f32)
            for sg in range(2):
                nc.vector.bn_stats(out=st[:nr, sg, :], in_=yc[:, sg, :])
            mv = stp.tile([128, 2], f32, tag="mv")
            nc.vector.bn_aggr(out=mv[:nr], in_=st[:nr])

            # rstd = 1/sqrt(var+eps) on DVE only (keeps ACT pure-gelu):
            # u0 = 0.5 + 0.5/(var+eps), two mult-only Newton steps
            # u <- u*(1.5 - 0.5*(var+eps)*u^2).  var+eps ~ 1 here so the
            # seed is within ~1% and two steps are ample.
            v = stp.tile([128, 1], f32, tag="v")
            nc.vector.tensor_scalar_add(out=v[:nr], in0=mv[:nr, 1:2], scalar1=EPS)
            u = stp.tile([128, 1], f32, tag="u")
            nc.vector.reciprocal(out=u[:nr], in_=v[:nr])
            nc.vector.tensor_scalar(
                out=u[:nr], in0=u[:nr], scalar1=0.5, scalar2=0.5,
                op0=ALU.mult, op1=ALU.add,
            )
            t = stp.tile([128, 1], f32, tag="t")
            for _ in range(2):
                nc.vector.tensor_mul(t[:nr], u[:nr], u[:nr])
                nc.vector.tensor_mul(t[:nr], t[:nr], v[:nr])
                nc.vector.tensor_scalar(
                    out=t[:nr], in0=t[:nr], scalar1=-0.5, scalar2=1.5,
                    op0=ALU.mult, op1=ALU.add,
                )
                nc.vector.tensor_mul(u[:nr], u[:nr], t[:nr])

            o_t = op.tile([128, C], bf16, tag="o")
            nc.vector.tensor_scalar(
                out=o_t[:nr],
                in0=y_t[:nr],
                scalar1=mv[:nr, 0:1],
                scalar2=u[:nr],
                op0=ALU.subtract,
                op1=ALU.mult,
            )
            if apply_gamma_beta:
                nc.vector.tensor_mul(o_t[:nr], o_t[:nr], gamma_rep[:nr])
                nc.vector.tensor_add(o_t[:nr], o_t[:nr], beta_rep[:nr])
            nc.sync.dma_start(out=y_d[n0 : n0 + nr, :], in_=o_t[:nr])

        prev = None
        for ic in range(NCHUNK):
            n0 = ic * PCH
            nr = min(PCH, RPC - n0)

            xb_t = xp.tile([128, C], bf16)
            nc.gpsimd.dma_start(out=xb_t[:nr], in_=xb_d[n0 : n0 + nr, :])

            G = gp.tile([128, RANK, CP], bf16)
            # G[p, r, c] viewed as [p, rh, r4, w, i]: r = 4*rh + r4,
            # c = 108*w + i  (matches the conv matmul column order).
            Gv = G.rearrange("p (rh r4) (w i) -> p rh r4 w i", rh=2, w=NW)
            dT = dp.tile([128, 2, 512], f32)

            for w in range(NW):
                H = hp.tile([128, 2, 512], f32)
                for rh in range(2):
                    nc.tensor.matmul(
                        H[:nr, rh, 0 : 4 * WW],
                        lhsT=xtw[w][:, n0 : n0 + nr],
                        rhs=band_t[:, rh, :],
                        start=True,
                        stop=True,
                    )
                h_view = H[:nr, :, 0 : 4 * WW].rearrange(
                    "p rh (r4 i) -> p rh r4 i", i=WW
                )
                nc.scalar.activation(
                    out=Gv[:nr, :, :, w, :], in_=h_view, func=AF.Gelu
                )

            # Software pipeline: previous chunk's down-projection + LN are
            # issued after this chunk's convs so ACT is never queue-blocked.
            if prev is not None:
                emit_tail(*prev)
            prev = (G, xb_t, dT, n0, nr)

        emit_tail(*prev)

    nc.compile()
    _CACHE[key] = nc
    return nc


def _host_prep(x, Wup, bup, Wdown, bdown, gamma, beta):
    """Build the per-core input maps (numpy only)."""
    bf = ml_dtypes.bfloat16
    f8 = ml_dtypes.float8_e4m3
    xf = np.ascontiguousarray(np.asarray(x, np.float32).reshape(ROWS, C))
    Wup_ = np.asarray(Wup, np.float32).reshape(RANK, KTAPS)
    bup_ = np.asarray(bup, np.float32).reshape(RANK)
    wd_ = np.asarray(Wdown, np.float32).reshape(RANK)
    gamma_ = np.asarray(gamma, np.float32).reshape(C)
    beta_ = np.asarray(beta, np.float32).reshape(C)

    # Transposed padded x [884, ROWS]: row p = xpad col p = x[:, p-3].
    xqt = np.zeros((884, ROWS), np.float32)
    xqt[3 : 3 + C, :] = xf.T
    # Window tap-slices, partition-major: xw[w, p, :] = xqt[108w + p],
    # with partition 127 = ones (bias row).
    xw = np.empty((NW, 128, ROWS), np.float32)
    for w in range(NW):
        xw[w, :127] = xqt[WW * w : WW * w + 127]
        xw[w, 127] = 1.0
    xw = xw.astype(bf)

    xb = xf.astype(bf)

    # Banded conv weights [tap, rh, r4*108 + i] (bf16):
    # band[i+k, rh, r4*WW+i] = Wup[4rh+r4, k]; band[127, rh, *] = bup.
    band = np.zeros((128, 2, 4 * WW), np.float32)
    i_idx = np.arange(WW)
    for r in range(RANK):
        rh, r4 = divmod(r, 4)
        for k in range(KTAPS):
            band[i_idx + k, rh, r4 * WW + i_idx] = Wup_[r, k]
        band[127, rh, r4 * WW : (r4 + 1) * WW] = bup_[r]
    band = band.astype(bf)

    # Scaled bf16 identities for the rank contraction.
    wdi = np.zeros((128, RANK, 128), np.float32)
    idx = np.arange(128)
    for r in range(RANK):
        wdi[idx, r, idx] = wd_[r]
    wdi = wdi.astype(bf)

    apply_gb = not (np.all(gamma_ == 1.0) and np.all(beta_ == 0.0))
    gb = np.stack([gamma_, beta_]).astype(np.float32)

    in_maps = []
    for i in range(NCORES):
        m = {
            "xw": np.ascontiguousarray(xw[:, :, i * RPC : (i + 1) * RPC]),
            "xb": xb[i * RPC : (i + 1) * RPC],
            "band": band,
            "wdi": wdi,
        }
        if apply_gb:
            m["gb"] = gb
        in_maps.append(m)
    return in_maps, apply_gb


def kernel(x, Wup, bup, Wdown, bdown, gamma, beta):
    from concourse.bass_utils import run_bass_kernel_spmd

    in_maps, apply_gb = _host_prep(x, Wup, bup, Wdown, bdown, gamma, beta)
    nc = _build(apply_gb)
    res = run_bass_kernel_spmd(nc, in_maps, core_ids=list(range(NCORES)))
    y = np.concatenate([res.results[i]["y"] for i in range(NCORES)], axis=0)
    return np.ascontiguousarray(
        y.astype(np.float32).reshape(B, L, C)
    )


# revision 14
# speedup vs baseline: 1.0470x; 1.0470x over previous
"""Trainium2 Bass kernel for nn_CrossVariableMixingConv.

Reference computation (per row of x, B*L rows, C=862 channels):
    h   = conv1d(x, Wup, k=7, pad=3) + bup      # (RANK=8 channels)
    g   = gelu(h)  (erf-exact)
    d   = sum_r Wdown[r] * g[r] + bdown
    y   = LayerNorm(x + d) * gamma + beta       # LN over C

Sharding: pure data parallelism - the B*L = 11520 rows are split into 8
shards of 1440 rows, one per NeuronCore; the ~70 conv params are
replicated.

Per-core kernel structure (rows on partitions, chunks of 128 rows):
  - The k=7 conv is a bf16 matmul with the data stationary: lhsT = a
    host-pretransposed tap-slice [128 taps, rows] per window (8 windows
    of 108 output cols, partition-major in DRAM so each load is a clean
    2880B-per-line DMA), rhs = a banded weight matrix [128, 432]
    covering 4 ranks x 108 cols; a baked ones-row adds bup in-matmul.
  - Gelu (erf-exact LUT) runs on the Scalar engine out of PSUM writing
    fp8(e4m3) G in SBUF.  The Scalar engine is the critical path
    (1 elem/cycle/lane x 8 ranks x C), so everything else stays off it.
  - The rank contraction (Wdown) is 4 accumulating fp8 DoubleRow
    matmuls per half of C - each contracts TWO ranks per streamed
    column (out = sum_s W[:,s].T @ I[:,s]) against scaled fp8
    identities - halving the Tensor-engine column count.
  - The down-projection of chunk N is issued AFTER chunk N+1's conv
    matmuls (software pipelining), so the Scalar engine never waits for
    the PE queue to drain a down-projection.
  - Residual + LayerNorm run on the Vector engine in bf16
    (bn_stats/bn_aggr, mult-only Newton rstd, fused (y-mu)*rstd);
    results DMA out in bf16.

bdown is dropped: LayerNorm is invariant to a constant shift per row.
gamma/beta are applied only when not identity (ones/zeros here).
"""

import sys

for _p in ("/opt/trn_rl_repo",):
    if _p not in sys.path:
        sys.path.insert(0, _p)

import numpy as np
import ml_dtypes

B, L, C = 16, 720, 862
RANK, KTAPS = 8, 7
NCORES = 8
ROWS = B * L                 # 11520
RPC = ROWS // NCORES         # 1440 rows per core
PCH = 128                    # rows per chunk (partition dim)
NCHUNK = (RPC + PCH - 1) // PCH   # 12 (11 full + 1 of 32)
WW = 108                     # conv output columns per window
NW = 8                       # windows: 8*108 = 864 >= 862
CP = NW * WW                 # 864 padded output columns
CHALF = C // 2               # 431
EPS = 1e-5

_CACHE: dict = {}


def _build(apply_gamma_beta: bool):
    """Build + compile the per-core Bass program. Cached per flag."""
    key = ("nc", apply_gamma_beta)
    if key in _CACHE:
        return _CACHE[key]

    from contextlib import ExitStack

    import concourse.bacc as bacc
    import concourse.bass as bass
    import concourse.tile as tile
    from concourse import mybir

    f32 = mybir.dt.float32
    bf16 = mybir.dt.bfloat16
    fp8 = mybir.dt.float8e4
    AF = mybir.ActivationFunctionType
    ALU = mybir.AluOpType
    DR = mybir.MatmulPerfMode.DoubleRow

    nc = bacc.Bacc(
        "TRN2", target_bir_lowering=False, debug=False, num_devices=NCORES
    )

    xw_d = nc.dram_tensor("xw", [NW, 128, RPC], bf16, kind="ExternalInput").ap()
    xb_d = nc.dram_tensor("xb", [RPC, C], bf16, kind="ExternalInput").ap()
    band_d = nc.dram_tensor("band", [128, 2, 4 * WW], bf16, kind="ExternalInput").ap()
    wdi_d = nc.dram_tensor("wdi", [128, RANK, 128], bf16, kind="ExternalInput").ap()
    if apply_gamma_beta:
        gb_d = nc.dram_tensor("gb", [2, C], f32, kind="ExternalInput").ap()
    y_d = nc.dram_tensor("y", [RPC, C], bf16, kind="ExternalOutput").ap()

    with tile.TileContext(nc) as tc, ExitStack() as ctx:
        singles = ctx.enter_context(tc.tile_pool(name="singles", bufs=1))
        xp = ctx.enter_context(tc.tile_pool(name="xin", bufs=2))
        gp = ctx.enter_context(tc.tile_pool(name="g", bufs=3))
        op = ctx.enter_context(tc.tile_pool(name="o", bufs=3))
        stp = ctx.enter_context(tc.tile_pool(name="st", bufs=3))
        hp = ctx.enter_context(tc.tile_pool(name="hps", bufs=3, space="PSUM"))
        dp = ctx.enter_context(tc.tile_pool(name="dps", bufs=1, space="PSUM"))

        # Tiny weight tensors first so the first conv matmul waits only
        # on its own window slice, not the whole input stream.
        band_t = singles.tile([128, 2, 4 * WW], bf16)
        nc.sync.dma_start(out=band_t, in_=band_d)
        wdi_t = singles.tile([128, RANK, 128], bf16)
        nc.gpsimd.dma_start(out=wdi_t, in_=wdi_d)
        # Window tap-slices [128 taps, rows], partition-major loads
        # (2880B per line); partition 127 is the baked ones row.
        # All chunk-0 slices first, then the bulk, so the first chunk's
        # full window sweep is fed after ~0.5MB instead of 2.95MB.
        xtw = []
        for w in range(NW):
            xtw_t = singles.tile([128, RPC], bf16, tag=f"xtw{w}")
            xtw.append(xtw_t)
        for w in range(NW):
            eng = nc.sync if w % 2 == 0 else nc.gpsimd
            eng.dma_start(out=xtw[w][:, 0:PCH], in_=xw_d[w, :, 0:PCH])
        for w in range(NW):
            eng = nc.sync if w % 2 == 0 else nc.gpsimd
            eng.dma_start(out=xtw[w][:, PCH:RPC], in_=xw_d[w, :, PCH:RPC])
        if apply_gamma_beta:
            gamma_rep = singles.tile([128, C], f32)
            beta_rep = singles.tile([128, C], f32)
            for rep, row in ((gamma_rep, 0), (beta_rep, 1)):
                src = bass.AP(
                    tensor=gb_d.tensor,
                    offset=gb_d.offset + row * C,
                    ap=[[0, 128], [1, C]],
                )
                nc.gpsimd.dma_start(out=rep, in_=src)

        def emit_tail(G, xb_t, dT, n0, nr):
            """Down-projection + residual/LN + store for a finished chunk.

            r-outer so each wdi_r stationary is loaded once (2 matmuls
            per load) - per-matmul LDWEIGHTS is the dominant PE tax."""
            for r in range(RANK):
                for hh in range(2):
                    nc.tensor.matmul(
                        dT[:nr, hh, 0:CHALF],
                        lhsT=wdi_t[:nr, r, :nr],
                        rhs=G[:nr, r, hh * CHALF : (hh + 1) * CHALF],
                        start=(r == 0),
                        stop=(r == RANK - 1),
                    )

            y_t = op.tile([128, C], bf16, tag="y")
            yc = y_t[:nr].rearrange("p (s c) -> p s c", s=2)
            nc.vector.tensor_add(
                out=yc,
                in0=xb_t[:nr].rearrange("p (s c) -> p s c", s=2),
                in1=dT[:nr, :, 0:CHALF],
            )

            st = stp.tile([128, 2, 6], f32)
            for sg in range(2):
                nc.vector.bn_stats(out=st[:nr, sg, :], in_=yc[:, sg, :])
            mv = stp.tile([128, 2], f32, tag="mv")
            nc.vector.bn_aggr(out=mv[:nr], in_=st[:nr])

            # rstd = 1/sqrt(var+eps) on DVE only (keeps ACT pure-gelu):
            # u0 = 0.5 + 0.5/(var+eps), two mult-only Newton steps
            # u <- u*(1.5 - 0.5*(var+eps)*u^2).  var+eps ~ 1 here so the
            # seed is within ~1% and two steps are ample.
            v = stp.tile([128, 1], f32, tag="v")
            nc.vector.tensor_scalar_add(out=v[:nr], in0=mv[:nr, 1:2], scalar1=EPS)
            u = stp.tile([128, 1], f32, tag="u")
            nc.vector.reciprocal(out=u[:nr], in_=v[:nr])
            nc.vector.tensor_scalar(
                out=u[:nr], in0=u[:nr], scalar1=0.5, scalar2=0.5,
                op0=ALU.mult, op1=ALU.add,
            )
            t = stp.tile([128, 1], f32, tag="t")
            for _ in range(2):
                nc.vector.tensor_mul(t[:nr], u[:nr], u[:nr])
                nc.vector.tensor_mul(t[:nr], t[:nr], v[:nr])
                nc.vector.tensor_scalar(
                    out=t[:nr], in0=t[:nr], scalar1=-0.5, scalar2=1.5,
                    op0=ALU.mult, op1=ALU.add,
                )
                nc.vector.tensor_mul(u[:nr], u[:nr], t[:nr])

            o_t = op.tile([128, C], bf16, tag="o")
            nc.vector.tensor_scalar(
                out=o_t[:nr],
                in0=y_t[:nr],
                scalar1=mv[:nr, 0:1],
                scalar2=u[:nr],
                op0=ALU.subtract,
                op1=ALU.mult,
            )
            if apply_gamma_beta:
                nc.vector.tensor_mul(o_t[:nr], o_t[:nr], gamma_rep[:nr])
                nc.vector.tensor_add(o_t[:nr], o_t[:nr], beta_rep[:nr])
            nc.sync.dma_start(out=y_d[n0 : n0 + nr, :], in_=o_t[:nr])

        prev = None
        for ic in range(NCHUNK):
            n0 = ic * PCH
            nr = min(PCH, RPC - n0)

            xb_t = xp.tile([128, C], bf16)
            nc.gpsimd.dma_start(out=xb_t[:nr], in_=xb_d[n0 : n0 + nr, :])

            G = gp.tile([128, RANK, CP], bf16)
            # G[p, r, c] viewed as [p, rh, r4, w, i]: r = 4*rh + r4,
            # c = 108*w + i  (matches the conv matmul column order).
            Gv = G.rearrange("p (rh r4) (w i) -> p rh r4 w i", rh=2, w=NW)
            dT = dp.tile([128, 2, 512], f32)

            for w in range(NW):
                H = hp.tile([128, 2, 512], f32)
                for rh in range(2):
                    nc.tensor.matmul(
                        H[:nr, rh, 0 : 4 * WW],
                        lhsT=xtw[w][:, n0 : n0 + nr],
                        rhs=band_t[:, rh, :],
                        start=True,
                        stop=True,
                    )
                h_view = H[:nr, :, 0 : 4 * WW].rearrange(
                    "p rh (r4 i) -> p rh r4 i", i=WW
                )
                nc.scalar.activation(
                    out=Gv[:nr, :, :, w, :], in_=h_view, func=AF.Gelu
                )

            # Software pipeline: previous chunk's down-projection + LN are
            # issued after this chunk's convs so ACT is never queue-blocked.
            if prev is not None:
                emit_tail(*prev)
            prev = (G, xb_t, dT, n0, nr)

        emit_tail(*prev)

    nc.compile()
    _CACHE[key] = nc
    return nc


def _host_prep(x, Wup, bup, Wdown, bdown, gamma, beta):
    """Build the per-core input maps (numpy only)."""
    bf = ml_dtypes.bfloat16
    f8 = ml_dtypes.float8_e4m3
    xf = np.ascontiguousarray(np.asarray(x, np.float32).reshape(ROWS, C))
    Wup_ = np.asarray(Wup, np.float32).reshape(RANK, KTAPS)
    bup_ = np.asarray(bup, np.float32).reshape(RANK)
    wd_ = np.asarray(Wdown, np.float32).reshape(RANK)
    gamma_ = np.asarray(gamma, np.float32).reshape(C)
    beta_ = np.asarray(beta, np.float32).reshape(C)

    # Transposed padded x [884, ROWS]: row p = xpad col p = x[:, p-3].
    xqt = np.zeros((884, ROWS), np.float32)
    xqt[3 : 3 + C, :] = xf.T
    # Window tap-slices, partition-major: xw[w, p, :] = xqt[108w + p],
    # with partition 127 = ones (bias row).
    xw = np.empty((NW, 128, ROWS), np.float32)
    for w in range(NW):
        xw[w, :127] = xqt[WW * w : WW * w + 127]
        xw[w, 127] = 1.0
    xw = xw.astype(bf)

    xb = xf.astype(bf)

    # Banded conv weights [tap, rh, r4*108 + i] (bf16):
    # band[i+k, rh, r4*WW+i] = Wup[4rh+r4, k]; band[127, rh, *] = bup.
    band = np.zeros((128, 2, 4 * WW), np.float32)
    i_idx = np.arange(WW)
    for r in range(RANK):
        rh, r4 = divmod(r, 4)
        for k in range(KTAPS):
            band[i_idx + k, rh, r4 * WW + i_idx] = Wup_[r, k]
        band[127, rh, r4 * WW : (r4 + 1) * WW] = bup_[r]
    band = band.astype(bf)

    # Scaled bf16 identities for the rank contraction.
    wdi = np.zeros((128, RANK, 128), np.float32)
    idx = np.arange(128)
    for r in range(RANK):
        wdi[idx, r, idx] = wd_[r]
    wdi = wdi.astype(bf)

    apply_gb = not (np.all(gamma_ == 1.0) and np.all(beta_ == 0.0))
    gb = np.stack([gamma_, beta_]).astype(np.float32)

    in_maps = []
    for i in range(NCORES):
        m = {
            "xw": np.ascontiguousarray(xw[:, :, i * RPC : (i + 1) * RPC]),
            "xb": xb[i * RPC : (i + 1) * RPC],
            "band": band,
            "wdi": wdi,
        }
        if apply_gb:
            m["gb"] = gb
        in_maps.append(m)
    return in_maps, apply_gb


def kernel(x, Wup, bup, Wdown, bdown, gamma, beta):
    from concourse.bass_utils import run_bass_kernel_spmd

    in_maps, apply_gb = _host_prep(x, Wup, bup, Wdown, bdown, gamma, beta)
    nc = _build(apply_gb)
    res = run_bass_kernel_spmd(nc, in_maps, core_ids=list(range(NCORES)))
    y = np.concatenate([res.results[i]["y"] for i in range(NCORES)], axis=0)
    return np.ascontiguousarray(
        y.astype(np.float32).reshape(B, L, C)
    )
